# revision 9
# baseline (speedup 1.0000x reference)
"""Trainium2 Bass kernel for CellPathwayAttentionAggregator (segment-reduce).

Math: out[b, s] = sum_{i in set s} softmax_s(attn_logits)[i] * G[b, flat_idx[i]]

Device decomposition (per core, transposed output):
    out^T = (W_exp^T @ G^T) * (1 / denom)[:, None]
where W_exp[g, s] = sum_{i in set s, flat_idx[i]=g} exp(attn_logits[i]) is the
(unnormalized) sparse aggregation matrix, scattered on the host as pure layout
prep (elementwise exp + scatter; no reductions on host), and
    denom[s] = sum_{i in set s} exp(attn_logits[i])
is computed ON DEVICE from a sets-on-partitions padded logits tile (ACT exp ->
DVE free-axis reduce -> DVE reciprocal; no PE involvement), followed by an
on-device per-partition normalization of the matmul output. The host
transposes each core's (sets x batch) block during assembly.

Sharding: 8 cores = 2 batch groups (512 rows) x 4 set groups (512 sets).
Each core accumulates a (512 x 8192) @ (8192 x 512) bf16 matmul in fp32 PSUM
over 64 K-tiles (4 set-subtile PSUM banks, N=512 moving operand), with a
dependency-free PE warmup against the HAM clock-gate and input tiles streamed
as fused 256KB G^T|W DMAs alternating across both HWDGE issuers.
"""

import sys

if "/opt/trn_rl_repo" not in sys.path:
    sys.path.insert(0, "/opt/trn_rl_repo")

import ml_dtypes
import numpy as np

NUM_SETS = 2048
NUM_GENESETS = 8192
BATCH = 1024
N_CORES = 8
BG, SG = 2, 4  # batch groups x set groups (BG*SG == N_CORES)
B_C = BATCH // BG  # 512 batch rows per core
S_C = NUM_SETS // SG  # 512 sets per core
P = 128
K_TILES = NUM_GENESETS // P  # 64
M_TILES = B_C // P  # 4
PAD_SLOTS = 128  # >= MAX set size (120)
NEG_FILL = -87.0  # exp(-87) ~ 1.6e-38 ~ 0 in fp32

_PROGRAM_CACHE = {}
LAST_RESULTS = None  # BassKernelResults of the most recent run (for profiling)


def _build_program():
    import concourse.mybir as mybir
    from concourse import bacc
    from concourse.tile import TileContext

    f32 = mybir.dt.float32
    bf16 = mybir.dt.bfloat16

    nc = bacc.Bacc("TRN2", target_bir_lowering=False, debug=False)
    # fused per-K-tile input: [:, :, :B_C] = G^T tile, [:, :, B_C:] = W tile.
    # One DMA per K-tile keeps every matmul's sync-wait count at <=1 (the
    # S3 LDWEIGHTS encoding only has a single wait slot).
    gw_d = nc.dram_tensor("gw", [K_TILES, P, B_C + S_C], bf16, kind="ExternalInput")
    plog_d = nc.dram_tensor(
        "plog", [P, (S_C // P) * PAD_SLOTS], f32, kind="ExternalInput"
    )
    out_d = nc.dram_tensor("out", [S_C, B_C], f32, kind="ExternalOutput")

    with TileContext(nc) as tc:
        with (
            tc.tile_pool(name="const", bufs=1) as cpool,
            tc.tile_pool(name="gw", bufs=12) as gwpool,
            tc.tile_pool(name="outp", bufs=4) as opool,
            tc.tile_pool(name="ps", bufs=1, space="PSUM") as ppool,
        ):
            # --- PE warmup: dependency-free N=1 matmuls on the pre-barrier
            # const tile keep the HAM clock-gate busy from right after the
            # entry barrier, so it reaches 8/8 (2.4 GHz) before the real
            # stream starts.
            const_one = nc.const_aps.aps[(bf16, 1.0)]
            scratch_ps = ppool.tile([1, 1], f32, tag="scratch")
            for _ in range(64):
                nc.tensor.matmul(
                    scratch_ps[:], const_one, const_one, start=True, stop=True
                )

            # --- tile 0 split across BOTH HWDGE rings (G-half on SP, W-half
            # on ACT) so the first matmul's data lands ~1us sooner; emitted
            # before the exp so ACT's ring isn't blocked behind the plog wait
            gw0 = gwpool.tile([P, B_C + S_C], bf16, tag="gw", name="gw0")
            nc.sync.dma_start(out=gw0[:, 0:B_C], in_=gw_d[0, :, 0:B_C])
            nc.scalar.dma_start(
                out=gw0[:, B_C : B_C + S_C], in_=gw_d[0, :, B_C : B_C + S_C]
            )

            # --- denominator chain: sets live on the PARTITION axis, so it
            # needs no PE matmuls at all (ACT exp -> DVE free-axis reduce ->
            # DVE reciprocal), fully parallel to the matmul stream ---
            SUBT = S_C // P  # 4 set-subtiles of 128 sets
            plog_sb = cpool.tile([P, SUBT * PAD_SLOTS], f32, tag="plog")
            nc.gpsimd.dma_start(out=plog_sb[:], in_=plog_d[:, :])
            exp_sb = cpool.tile([P, SUBT * PAD_SLOTS], f32, tag="exp")
            nc.scalar.activation(
                exp_sb[:], plog_sb[:], mybir.ActivationFunctionType.Exp
            )
            den_sb = cpool.tile([P, SUBT], f32, tag="den")
            nc.vector.tensor_reduce(
                out=den_sb[:],
                in_=exp_sb[:].rearrange("p (j t) -> p j t", t=PAD_SLOTS),
                op=mybir.AluOpType.add,
                axis=mybir.AxisListType.X,
            )
            recip_sb = cpool.tile([P, SUBT], f32, tag="recip")
            nc.vector.reciprocal(recip_sb[:], den_sb[:])

            # --- main matmul: out^T = W_c^T @ G_c^T, accumulated over 64
            # K-tiles; output has sets on partitions, batch on free ---
            acc = [
                ppool.tile([P, B_C], f32, tag=f"acc{j}", name=f"acc{j}")
                for j in range(SUBT)
            ]
            for k in range(K_TILES):
                if k == 0:
                    gw_sb = gw0
                else:
                    gw_sb = gwpool.tile([P, B_C + S_C], bf16, tag="gw")
                    # alternate the two HWDGE issuers (SP + ACT) in steady
                    # state to halve per-ring FIFO pressure; keep early tiles
                    # on SP so the exp chain on ACT isn't stuck behind DMA
                    # slot-waits
                    dma_eng = nc.scalar if (k >= 16 and k % 2 == 1) else nc.sync
                    dma_eng.dma_start(out=gw_sb[:], in_=gw_d[k, :, :])
                for j in range(SUBT):
                    nc.tensor.matmul(
                        acc[j][:],
                        gw_sb[:, B_C + j * P : B_C + (j + 1) * P],
                        gw_sb[:, 0:B_C],
                        start=(k == 0),
                        stop=(k == K_TILES - 1),
                    )

            # --- normalize each output row by 1/denom (per-partition scalar)
            # and store; host transposes at assembly. Split across DVE and ACT
            # (activation Copy with a per-partition scale AP) so the four
            # scales run pairwise-parallel instead of serializing on DVE ---
            for j in range(SUBT):
                o_sb = opool.tile([P, B_C], f32, tag="osb")
                if j % 2 == 0:
                    nc.vector.tensor_scalar_mul(
                        o_sb[:], acc[j][:], recip_sb[:, j : j + 1]
                    )
                else:
                    nc.scalar.activation(
                        o_sb[:],
                        acc[j][:],
                        mybir.ActivationFunctionType.Copy,
                        bias=0.0,
                        scale=recip_sb[:, j : j + 1],
                    )
                nc.sync.dma_start(out=out_d[j * P : (j + 1) * P, :], in_=o_sb[:])

    nc.finalize()
    return nc


def _build_program_raw():
    """Raw-Bass pipeline with hand-placed semaphores — avoids the Tile/Bacc
    event-semaphore preamble (~7us) and exit butterfly (~8us) measured in the
    baseline trace (first matmul at t=7.2us; 13.2us tail after the last one).

    Same math/layout as the Tile version: sets-on-partitions everywhere, so
    the denominator chain (ACT exp -> DVE segmented reduce -> DVE reciprocal)
    never touches the PE, and the final normalize is a per-partition scalar
    (DVE tensor_scalar for j=0,2; ACT Copy-with-scale for j=1,3) feeding four
    output DMAs split across two rings.

    Sem plan:
      s_slot[j]: +16 per gw-tile DMA landing in slot j (tile 0's two half-DMAs
                 inc +8 each); PE waits 16*(wrap+1) before tile k's matmuls
      s_plog:    +16 when the padded-logits DMA (gpsimd ring) lands
      s_exp:     +1 by ACT when the exp tile is ready
      s_recip:   +1 by DVE after the reciprocal
      s_mm:      +1 by PE per finished gw tile (backpressures both DMA rings)
      s_fin:     +1 by PE drain after the last matmul (PSUM writeback flushed)
      s_norm[j]: +1 when output subtile j is normalized into o_sb
      s_done:    +16 per output DMA (final completion wait on gpsimd)
    """
    import concourse.bass as bass
    import concourse.mybir as mybir

    f32 = mybir.dt.float32
    bf16 = mybir.dt.bfloat16
    FD = B_C + S_C  # fused free dim: 1024
    BUFS = 16
    SUBT = S_C // P  # 4
    WARMUP = 64

    nc = bass.Bass("TRN2")
    gw_d = nc.dram_tensor("gw", [K_TILES, P, FD], bf16, kind="ExternalInput")
    plog_d = nc.dram_tensor("plog", [P, SUBT, PAD_SLOTS], f32, kind="ExternalInput")
    out_d = nc.dram_tensor("out", [S_C, B_C], f32, kind="ExternalOutput")

    from contextlib import ExitStack

    with ExitStack() as ctx:
        gw_sb = ctx.enter_context(nc.sbuf_tensor([P, BUFS, FD], bf16))
        plog_sb = ctx.enter_context(nc.sbuf_tensor([P, SUBT, PAD_SLOTS], f32))
        exp_sb = ctx.enter_context(nc.sbuf_tensor([P, SUBT, PAD_SLOTS], f32))
        den_sb = ctx.enter_context(nc.sbuf_tensor([P, SUBT], f32))
        recip_sb = ctx.enter_context(nc.sbuf_tensor([P, SUBT], f32))
        o_sb = ctx.enter_context(nc.sbuf_tensor([P, SUBT, B_C], f32))
        acc_ps = ctx.enter_context(nc.psum_tensor([P, SUBT, B_C], f32))
        scratch_ps = ctx.enter_context(nc.psum_tensor([1, 1], f32))
        s_slot = [
            ctx.enter_context(nc.semaphore(name=f"s_slot{j}")) for j in range(BUFS)
        ]
        s_norm = [
            ctx.enter_context(nc.semaphore(name=f"s_norm{j}")) for j in range(SUBT)
        ]
        s_plog = ctx.enter_context(nc.semaphore(name="s_plog"))
        s_exp = ctx.enter_context(nc.semaphore(name="s_exp"))
        s_den = ctx.enter_context(nc.semaphore(name="s_den"))
        s_recip = ctx.enter_context(nc.semaphore(name="s_recip"))
        s_mm = ctx.enter_context(nc.semaphore(name="s_mm"))
        s_fin = ctx.enter_context(nc.semaphore(name="s_fin"))
        s_done = ctx.enter_context(nc.semaphore(name="s_done"))
        block = ctx.enter_context(nc.Block(no_gpsimd_drain=True))

        @block.sync
        def _(sync):
            # tile 0's G^T half on this ring; W half on the ACT ring so the
            # first matmul's data lands as early as possible
            sync.dma_start(gw_sb[:, 0, 0:B_C], gw_d[0, :, 0:B_C]).then_inc(
                s_slot[0], 16
            )
            for k in range(2, K_TILES, 2):
                if k >= BUFS:
                    sync.wait_ge(s_mm, k - BUFS + 1)
                sync.dma_start(gw_sb[:, k % BUFS, :], gw_d[k, :, :]).then_inc(
                    s_slot[k % BUFS], 16
                )
            for j in (0, 2):
                sync.wait_ge(s_norm[j], 1)
                sync.dma_start(out_d[j * P : (j + 1) * P, :], o_sb[:, j, :]).then_inc(
                    s_done, 16
                )

        @block.scalar
        def _(scalar):
            scalar.dma_start(gw_sb[:, 0, B_C:FD], gw_d[0, :, B_C:FD]).then_inc(
                s_slot[0], 16
            )
            # a few tiles up front so the PE never starves while exp runs
            for k in (1, 3, 5):
                scalar.dma_start(gw_sb[:, k % BUFS, :], gw_d[k, :, :]).then_inc(
                    s_slot[k % BUFS], 16
                )
            scalar.wait_ge(s_plog, 16)
            scalar.activation(
                exp_sb[:], plog_sb[:], mybir.ActivationFunctionType.Exp
            ).then_inc(s_exp, 1)
            for k in range(7, K_TILES, 2):
                if k >= BUFS:
                    scalar.wait_ge(s_mm, k - BUFS + 1)
                scalar.dma_start(gw_sb[:, k % BUFS, :], gw_d[k, :, :]).then_inc(
                    s_slot[k % BUFS], 16
                )
            scalar.wait_ge(s_recip, 1)
            scalar.wait_ge(s_fin, 1)
            for j in (1, 3):
                scalar.activation(
                    o_sb[:, j, :],
                    acc_ps[:, j, :],
                    mybir.ActivationFunctionType.Copy,
                    bias=0.0,
                    scale=recip_sb[:, j : j + 1],
                ).then_inc(s_norm[j], 1)
            for j in (1, 3):
                scalar.wait_ge(s_norm[j], 1)
                scalar.dma_start(
                    out_d[j * P : (j + 1) * P, :], o_sb[:, j, :]
                ).then_inc(s_done, 16)

        @block.tensor
        def _(tensor):
            # dependency-free warmups on the pre-barrier const tile keep the
            # HAM clock-gate busy so the PE reaches 8/8 before the stream
            const_one = nc.const_aps.aps[(bf16, 1.0)]
            for _ in range(WARMUP):
                tensor.matmul(
                    scratch_ps[:], const_one, const_one, start=True, stop=True
                )
            for k in range(K_TILES):
                # slot 0 gets +32 for tile 0 (two half-DMAs at +16 each)
                bias = 1 if k % BUFS == 0 else 0
                tensor.wait_ge(s_slot[k % BUFS], 16 * (k // BUFS + 1 + bias))
                tile = gw_sb[:, k % BUFS, :]
                for j in range(SUBT):
                    mm = tensor.matmul(
                        acc_ps[:, j, :],
                        tile[:, B_C + j * P : B_C + (j + 1) * P],
                        tile[:, 0:B_C],
                        start=(k == 0),
                        stop=(k == K_TILES - 1),
                    )
                    if j == SUBT - 1:
                        # operands fully streamed at retire -> slot reusable
                        mm.then_inc(s_mm, 1)
            # drain flushes the PSUM writeback before DVE/ACT read acc
            tensor.drain().then_inc(s_fin, 1)

        @block.vector
        def _(vector):
            vector.wait_ge(s_exp, 1)
            vector.tensor_reduce(
                out=den_sb[:],
                in_=exp_sb[:],
                op=mybir.AluOpType.add,
                axis=mybir.AxisListType.X,
            ).then_inc(s_den, 1)
            # same-engine RAW still needs a sem edge (DVE pipelines insts)
            vector.wait_ge(s_den, 1)
            vector.reciprocal(recip_sb[:], den_sb[:]).then_inc(s_recip, 1)
            vector.wait_ge(s_recip, 1)
            vector.wait_ge(s_fin, 1)
            for j in (0, 2):
                vector.tensor_scalar_mul(
                    o_sb[:, j, :], acc_ps[:, j, :], recip_sb[:, j : j + 1]
                ).then_inc(s_norm[j], 1)

        @block.gpsimd
        def _(gpsimd):
            gpsimd.dma_start(plog_sb[:], plog_d[:, :, :]).then_inc(s_plog, 16)
            gpsimd.wait_ge(s_done, 16 * SUBT)

    nc.finalize()
    return nc


def _get_program():
    if "nc" not in _PROGRAM_CACHE:
        _PROGRAM_CACHE["nc"] = _build_program_raw()
    return _PROGRAM_CACHE["nc"]


def _ensure_ntff_hook():
    """Make NTFF profiling under axon work (BASS_TRACE=1): the image's antenv
    package lacks the axon_hooks holder module, so synthesize it and register
    the ctypes-based profile hook from trn_agent_boot. Best-effort."""
    import types

    try:
        import antenv

        try:
            from antenv.axon_hooks import get_axon_ntff_profile_hook  # noqa: F401

            return  # already present and registered
        except ImportError:
            pass
        mod = types.ModuleType("antenv.axon_hooks")
        _holder = [None]
        mod.set_axon_ntff_profile_hook = lambda h: _holder.__setitem__(0, h)
        mod.get_axon_ntff_profile_hook = lambda: _holder[0]
        sys.modules["antenv.axon_hooks"] = mod
        antenv.axon_hooks = mod

        from trn_agent_boot.trn_boot import _ntff_profile_via_ctypes

        hook = _ntff_profile_via_ctypes("/opt/axon/libaxon_pjrt.so")
        mod.set_axon_ntff_profile_hook(hook)
    except Exception:
        pass


def kernel(**inputs):
    global LAST_RESULTS
    G = np.asarray(inputs["geneset_features"], dtype=np.float32)
    logits = np.asarray(inputs["attn_logits"], dtype=np.float32)
    flat_idx = np.asarray(inputs["flat_idx"]).astype(np.int64)
    seg = np.asarray(inputs["segment_ids"]).astype(np.int64)
    T = logits.shape[0]

    # Host-side layout prep: scatter exp(logits) into the sparse aggregation
    # matrix (member sets are sampled without replacement, so (idx, seg) pairs
    # are unique within a set and the fancy assignment is collision-free).
    e32 = np.exp(logits)
    W = np.zeros((NUM_GENESETS, NUM_SETS), dtype=ml_dtypes.bfloat16)
    W[flat_idx, seg] = e32.astype(ml_dtypes.bfloat16)

    # Padded per-set logit columns; device computes denominators from these.
    sizes = np.bincount(seg, minlength=NUM_SETS)
    starts = np.concatenate([[0], np.cumsum(sizes)[:-1]])
    pos = np.arange(T) - starts[seg]
    plogT = np.full((PAD_SLOTS, NUM_SETS), NEG_FILL, dtype=np.float32)
    plogT[pos, seg] = logits

    Gb = G.astype(ml_dtypes.bfloat16)

    GbT = np.ascontiguousarray(Gb.T)  # (8192, 1024)
    in_maps = []
    for c in range(N_CORES):
        bg, sg = divmod(c, SG)
        gt = GbT[:, bg * B_C : (bg + 1) * B_C].reshape(K_TILES, P, B_C)
        w = W[:, sg * S_C : (sg + 1) * S_C].reshape(K_TILES, P, S_C)
        gw = np.concatenate([gt, w], axis=2)  # (K_TILES, P, B_C + S_C)
        # sets-on-partitions layout: plog[s_local, j*128+t] = logit slot t
        # of set (sg*S_C + j*128 + s_local)
        chunk = plogT[:, sg * S_C : (sg + 1) * S_C]  # (slots, S_C)
        plog = np.ascontiguousarray(
            chunk.reshape(PAD_SLOTS, S_C // P, P).transpose(2, 1, 0)
        )  # (P, SUBT, PAD_SLOTS)
        in_maps.append({"gw": np.ascontiguousarray(gw), "plog": plog})

    from concourse.bass_utils import run_bass_kernel_spmd

    _ensure_ntff_hook()
    nc = _get_program()
    res = run_bass_kernel_spmd(nc, in_maps, core_ids=list(range(N_CORES)))
    LAST_RESULTS = res

    out = np.empty((BATCH, NUM_SETS), dtype=np.float32)
    for c in range(N_CORES):
        bg, sg = divmod(c, SG)
        out[bg * B_C : (bg + 1) * B_C, sg * S_C : (sg + 1) * S_C] = res.results[c][
            "out"
        ].T
    return out



# revision 13
# speedup vs baseline: 1.0119x; 1.0119x over previous
"""Trainium2 Bass kernel for CellPathwayAttentionAggregator (segment-reduce).

Math: out[b, s] = sum_{i in set s} softmax_s(attn_logits)[i] * G[b, flat_idx[i]]

Device decomposition (per core, transposed output):
    out^T = (W_exp^T @ G^T) * (1 / denom)[:, None]
where W_exp[g, s] = sum_{i in set s, flat_idx[i]=g} exp(attn_logits[i]) is the
(unnormalized) sparse aggregation matrix, scattered on the host as pure layout
prep (elementwise exp + scatter; no reductions on host), and
    denom[s] = sum_{i in set s} exp(attn_logits[i])
is computed ON DEVICE from a sets-on-partitions padded logits tile (ACT exp ->
DVE free-axis reduce -> DVE reciprocal; no PE involvement), followed by an
on-device per-partition normalization of the matmul output. The host
transposes each core's (sets x batch) block during assembly.

Sharding: 8 cores = 2 batch groups (512 rows) x 4 set groups (512 sets).
Each core accumulates a (512 x 8192) @ (8192 x 512) bf16 matmul in fp32 PSUM
over 64 K-tiles (4 set-subtile PSUM banks, N=512 moving operand), with a
dependency-free PE warmup against the HAM clock-gate and input tiles streamed
as fused 256KB G^T|W DMAs alternating across both HWDGE issuers.
"""

import sys

if "/opt/trn_rl_repo" not in sys.path:
    sys.path.insert(0, "/opt/trn_rl_repo")

import ml_dtypes
import numpy as np

NUM_SETS = 2048
NUM_GENESETS = 8192
BATCH = 1024
N_CORES = 8
BG, SG = 2, 4  # batch groups x set groups (BG*SG == N_CORES)
B_C = BATCH // BG  # 512 batch rows per core
S_C = NUM_SETS // SG  # 512 sets per core
P = 128
K_TILES = NUM_GENESETS // P  # 64
M_TILES = B_C // P  # 4
PAD_SLOTS = 128  # >= MAX set size (120)
NEG_FILL = -87.0  # exp(-87) ~ 1.6e-38 ~ 0 in fp32

_PROGRAM_CACHE = {}
LAST_RESULTS = None  # BassKernelResults of the most recent run (for profiling)


def _build_program():
    import concourse.mybir as mybir
    from concourse import bacc
    from concourse.tile import TileContext

    f32 = mybir.dt.float32
    bf16 = mybir.dt.bfloat16

    nc = bacc.Bacc("TRN2", target_bir_lowering=False, debug=False)
    # fused per-K-tile input: [:, :, :B_C] = G^T tile, [:, :, B_C:] = W tile.
    # One DMA per K-tile keeps every matmul's sync-wait count at <=1 (the
    # S3 LDWEIGHTS encoding only has a single wait slot).
    gw_d = nc.dram_tensor("gw", [K_TILES, P, B_C + S_C], bf16, kind="ExternalInput")
    plog_d = nc.dram_tensor(
        "plog", [P, (S_C // P) * PAD_SLOTS], f32, kind="ExternalInput"
    )
    out_d = nc.dram_tensor("out", [S_C, B_C], f32, kind="ExternalOutput")

    with TileContext(nc) as tc:
        with (
            tc.tile_pool(name="const", bufs=1) as cpool,
            tc.tile_pool(name="gw", bufs=12) as gwpool,
            tc.tile_pool(name="outp", bufs=4) as opool,
            tc.tile_pool(name="ps", bufs=1, space="PSUM") as ppool,
        ):
            # --- PE warmup: dependency-free N=1 matmuls on the pre-barrier
            # const tile keep the HAM clock-gate busy from right after the
            # entry barrier, so it reaches 8/8 (2.4 GHz) before the real
            # stream starts.
            const_one = nc.const_aps.aps[(bf16, 1.0)]
            scratch_ps = ppool.tile([1, 1], f32, tag="scratch")
            for _ in range(64):
                nc.tensor.matmul(
                    scratch_ps[:], const_one, const_one, start=True, stop=True
                )

            # --- tile 0 split across BOTH HWDGE rings (G-half on SP, W-half
            # on ACT) so the first matmul's data lands ~1us sooner; emitted
            # before the exp so ACT's ring isn't blocked behind the plog wait
            gw0 = gwpool.tile([P, B_C + S_C], bf16, tag="gw", name="gw0")
            nc.sync.dma_start(out=gw0[:, 0:B_C], in_=gw_d[0, :, 0:B_C])
            nc.scalar.dma_start(
                out=gw0[:, B_C : B_C + S_C], in_=gw_d[0, :, B_C : B_C + S_C]
            )

            # --- denominator chain: sets live on the PARTITION axis, so it
            # needs no PE matmuls at all (ACT exp -> DVE free-axis reduce ->
            # DVE reciprocal), fully parallel to the matmul stream ---
            SUBT = S_C // P  # 4 set-subtiles of 128 sets
            plog_sb = cpool.tile([P, SUBT * PAD_SLOTS], f32, tag="plog")
            nc.gpsimd.dma_start(out=plog_sb[:], in_=plog_d[:, :])
            exp_sb = cpool.tile([P, SUBT * PAD_SLOTS], f32, tag="exp")
            nc.scalar.activation(
                exp_sb[:], plog_sb[:], mybir.ActivationFunctionType.Exp
            )
            den_sb = cpool.tile([P, SUBT], f32, tag="den")
            nc.vector.tensor_reduce(
                out=den_sb[:],
                in_=exp_sb[:].rearrange("p (j t) -> p j t", t=PAD_SLOTS),
                op=mybir.AluOpType.add,
                axis=mybir.AxisListType.X,
            )
            recip_sb = cpool.tile([P, SUBT], f32, tag="recip")
            nc.vector.reciprocal(recip_sb[:], den_sb[:])

            # --- main matmul: out^T = W_c^T @ G_c^T, accumulated over 64
            # K-tiles; output has sets on partitions, batch on free ---
            acc = [
                ppool.tile([P, B_C], f32, tag=f"acc{j}", name=f"acc{j}")
                for j in range(SUBT)
            ]
            for k in range(K_TILES):
                if k == 0:
                    gw_sb = gw0
                else:
                    gw_sb = gwpool.tile([P, B_C + S_C], bf16, tag="gw")
                    # alternate the two HWDGE issuers (SP + ACT) in steady
                    # state to halve per-ring FIFO pressure; keep early tiles
                    # on SP so the exp chain on ACT isn't stuck behind DMA
                    # slot-waits
                    dma_eng = nc.scalar if (k >= 16 and k % 2 == 1) else nc.sync
                    dma_eng.dma_start(out=gw_sb[:], in_=gw_d[k, :, :])
                for j in range(SUBT):
                    nc.tensor.matmul(
                        acc[j][:],
                        gw_sb[:, B_C + j * P : B_C + (j + 1) * P],
                        gw_sb[:, 0:B_C],
                        start=(k == 0),
                        stop=(k == K_TILES - 1),
                    )

            # --- normalize each output row by 1/denom (per-partition scalar)
            # and store; host transposes at assembly. Split across DVE and ACT
            # (activation Copy with a per-partition scale AP) so the four
            # scales run pairwise-parallel instead of serializing on DVE ---
            for j in range(SUBT):
                o_sb = opool.tile([P, B_C], f32, tag="osb")
                if j % 2 == 0:
                    nc.vector.tensor_scalar_mul(
                        o_sb[:], acc[j][:], recip_sb[:, j : j + 1]
                    )
                else:
                    nc.scalar.activation(
                        o_sb[:],
                        acc[j][:],
                        mybir.ActivationFunctionType.Copy,
                        bias=0.0,
                        scale=recip_sb[:, j : j + 1],
                    )
                nc.sync.dma_start(out=out_d[j * P : (j + 1) * P, :], in_=o_sb[:])

    nc.finalize()
    return nc


def _build_program_raw():
    """Raw-Bass pipeline with hand-placed semaphores.

    Trace findings this version addresses (vs the Tile baseline):
      - per-DMA-queue throughput is ~130 GB/s at 2KB descriptor rows (~10ns
        fixed cost per row): gw is now a FLAT [P, K*FD] DRAM tensor so one
        DMA moves 1-2 K-tiles as 2-4KB contiguous rows, and the two HWDGE
        rings (SP + ACT) alternate groups -> ~380 GB/s aggregate > the PE's
        300 GB/s steady-state demand.
      - ring completion is IN-ORDER, so one counting semaphore per ring
        suffices for the PE's per-group waits.
      - plog rides the SP ring LAST (the swdge path took 3.4us and stalled
        the exp); exp runs on ACT after all its gw issues (the 1.3us
        ACT_TABLE_LOAD then hurts nothing).
      - output is normalized into bf16 (host upcasts) and DMAd as four
        128KB stores alternating rings as each subtile's normalize lands.
      - 96 dependency-free warmup matmuls cover the first-tile DMA latency
        (~2.8us incl. the 900ns sem propagation) and hold the PE clock up.

    Sem plan:
      s_sp / s_act: +16 per DMA landing on that ring (in-order per ring);
                    PE waits 16*cum_count(group) before a group's matmuls
      s_mm:         +1 by PE per finished gw tile (backpressures both rings)
      s_plog/s_exp/s_den/s_recip: denominator chain ordering
      s_fin:        +1 by PE drain after the last matmul
      s_norm[j]:    +1 when output subtile j is normalized into o_sb
      s_done:       +16 per output DMA (final wait on gpsimd)
    """
    import concourse.bass as bass
    import concourse.mybir as mybir

    f32 = mybir.dt.float32
    bf16 = mybir.dt.bfloat16
    FD = B_C + S_C  # fused free dim per K-tile: 1024
    NBUF = 33  # tile slots; 33 so no 2-tile group straddles the wrap
    SUBT = S_C // P  # 4
    WARMUP = 96
    SP, ACT = 0, 1

    # DMA groups: (k0, k1, ring). Singles for tiles 0(G/W-half),1,2 to get
    # the PE started at fine granularity, pairs (4KB rows) for the rest.
    groups = []
    groups.append((0, 1, SP))  # tile 0 G half (cols 0:B_C)
    groups.append((0, 1, ACT))  # tile 0 W half (cols B_C:FD)
    groups.append((1, 2, SP))
    groups.append((2, 3, ACT))
    ring = SP
    for k0 in range(3, 62, 2):
        groups.append((k0, k0 + 2, ring))
        ring = 1 - ring
    groups.append((63, 64, SP))

    # one semaphore per group: ring completion is NOT in-order (16 DMA
    # engines pull descriptors concurrently), so counting sems are ambiguous
    head_waits = {}  # k0 -> list of group indices
    for gi, (k0, k1, r) in enumerate(groups):
        head_waits.setdefault(k0, []).append(gi)

    nc = bass.Bass("TRN2")
    gw_d = nc.dram_tensor("gw", [P, K_TILES * FD], bf16, kind="ExternalInput")
    plog_d = nc.dram_tensor("plog", [P, SUBT, PAD_SLOTS], f32, kind="ExternalInput")
    out_d = nc.dram_tensor("out", [P, SUBT, B_C], bf16, kind="ExternalOutput")

    from contextlib import ExitStack

    with ExitStack() as ctx:
        gw_sb = ctx.enter_context(nc.sbuf_tensor([P, NBUF * FD], bf16))
        plog_sb = ctx.enter_context(nc.sbuf_tensor([P, SUBT, PAD_SLOTS], f32))
        exp_sb = ctx.enter_context(nc.sbuf_tensor([P, SUBT, PAD_SLOTS], f32))
        den_sb = ctx.enter_context(nc.sbuf_tensor([P, SUBT], f32))
        recip_sb = ctx.enter_context(nc.sbuf_tensor([P, SUBT], f32))
        o_sb = ctx.enter_context(nc.sbuf_tensor([P, SUBT, B_C], bf16))
        acc_ps = ctx.enter_context(nc.psum_tensor([P, SUBT, B_C], f32))
        scratch_ps = ctx.enter_context(nc.psum_tensor([1, 1], f32))
        s_grp = [
            ctx.enter_context(nc.semaphore(name=f"s_grp{gi}"))
            for gi in range(len(groups))
        ]
        s_norm = [
            ctx.enter_context(nc.semaphore(name=f"s_norm{j}")) for j in range(SUBT)
        ]
        s_plog = ctx.enter_context(nc.semaphore(name="s_plog"))
        s_exp = ctx.enter_context(nc.semaphore(name="s_exp"))
        s_den = ctx.enter_context(nc.semaphore(name="s_den"))
        s_recip = ctx.enter_context(nc.semaphore(name="s_recip"))
        s_mm = ctx.enter_context(nc.semaphore(name="s_mm"))
        s_fin = ctx.enter_context(nc.semaphore(name="s_fin"))
        s_done = ctx.enter_context(nc.semaphore(name="s_done"))
        block = ctx.enter_context(nc.Block(no_gpsimd_drain=True))

        def emit_gw_dmas(eng, my_ring):
            for gi, (k0, k1, r) in enumerate(groups):
                if r != my_ring:
                    continue
                if k1 > NBUF:
                    eng.wait_ge(s_mm, k1 - NBUF)
                slot = k0 % NBUF
                if k0 == 0:  # half-tile DMAs for tile 0
                    c0, c1 = (0, B_C) if my_ring == SP else (B_C, FD)
                    eng.dma_start(
                        gw_sb[:, slot * FD + c0 : slot * FD + c1],
                        gw_d[:, k0 * FD + c0 : k0 * FD + c1],
                    ).then_inc(s_grp[gi], 16)
                else:
                    eng.dma_start(
                        gw_sb[:, slot * FD : (slot + k1 - k0) * FD],
                        gw_d[:, k0 * FD : k1 * FD],
                    ).then_inc(s_grp[gi], 16)

        @block.sync
        def _(sync):
            emit_gw_dmas(sync, SP)
            sync.dma_start(plog_sb[:], plog_d[:, :, :]).then_inc(s_plog, 16)
            for j in (0, 2):
                sync.wait_ge(s_norm[j], 1)
                sync.dma_start(out_d[:, j, :], o_sb[:, j, :]).then_inc(s_done, 16)

        @block.scalar
        def _(scalar):
            emit_gw_dmas(scalar, ACT)
            scalar.wait_ge(s_plog, 16)
            scalar.activation(
                exp_sb[:], plog_sb[:], mybir.ActivationFunctionType.Exp
            ).then_inc(s_exp, 1)
            scalar.wait_ge(s_recip, 1)
            scalar.wait_ge(s_fin, 1)
            for j in (1, 3):
                scalar.activation(
                    o_sb[:, j, :],
                    acc_ps[:, j, :],
                    mybir.ActivationFunctionType.Copy,
                    bias=0.0,
                    scale=recip_sb[:, j : j + 1],
                ).then_inc(s_norm[j], 1)
            for j in (1, 3):
                scalar.wait_ge(s_norm[j], 1)
                scalar.dma_start(out_d[:, j, :], o_sb[:, j, :]).then_inc(s_done, 16)

        @block.tensor
        def _(tensor):
            # dependency-free warmups on the pre-barrier const tile keep the
            # HAM clock-gate busy so the PE reaches 8/8 before the stream
            const_one = nc.const_aps.aps[(bf16, 1.0)]
            for _ in range(WARMUP):
                tensor.matmul(
                    scratch_ps[:], const_one, const_one, start=True, stop=True
                )
            for k in range(K_TILES):
                for gi in head_waits.get(k, ()):
                    tensor.wait_ge(s_grp[gi], 16)
                slot = k % NBUF
                tile = gw_sb[:, slot * FD : (slot + 1) * FD]
                for j in range(SUBT):
                    mm = tensor.matmul(
                        acc_ps[:, j, :],
                        tile[:, B_C + j * P : B_C + (j + 1) * P],
                        tile[:, 0:B_C],
                        start=(k == 0),
                        stop=(k == K_TILES - 1),
                    )
                    if j == SUBT - 1:
                        # operands fully streamed at retire -> slot reusable
                        mm.then_inc(s_mm, 1)
            # drain flushes the PSUM writeback before DVE/ACT read acc
            tensor.drain().then_inc(s_fin, 1)

        @block.vector
        def _(vector):
            vector.wait_ge(s_exp, 1)
            vector.tensor_reduce(
                out=den_sb[:],
                in_=exp_sb[:],
                op=mybir.AluOpType.add,
                axis=mybir.AxisListType.X,
            ).then_inc(s_den, 1)
            # same-engine RAW still needs a sem edge (DVE pipelines insts)
            vector.wait_ge(s_den, 1)
            vector.reciprocal(recip_sb[:], den_sb[:]).then_inc(s_recip, 1)
            vector.wait_ge(s_recip, 1)
            vector.wait_ge(s_fin, 1)
            for j in (0, 2):
                vector.tensor_scalar_mul(
                    o_sb[:, j, :], acc_ps[:, j, :], recip_sb[:, j : j + 1]
                ).then_inc(s_norm[j], 1)

        @block.gpsimd
        def _(gpsimd):
            gpsimd.wait_ge(s_done, 16 * SUBT)

    nc.finalize()
    return nc


def _get_program():
    if "nc" not in _PROGRAM_CACHE:
        _PROGRAM_CACHE["nc"] = _build_program_raw()
    return _PROGRAM_CACHE["nc"]


def _ensure_ntff_hook():
    """Make NTFF profiling under axon work (BASS_TRACE=1): the image's antenv
    package lacks the axon_hooks holder module, so synthesize it and register
    the ctypes-based profile hook from trn_agent_boot. Best-effort."""
    import types

    try:
        import antenv

        try:
            from antenv.axon_hooks import get_axon_ntff_profile_hook  # noqa: F401

            return  # already present and registered
        except ImportError:
            pass
        mod = types.ModuleType("antenv.axon_hooks")
        _holder = [None]
        mod.set_axon_ntff_profile_hook = lambda h: _holder.__setitem__(0, h)
        mod.get_axon_ntff_profile_hook = lambda: _holder[0]
        sys.modules["antenv.axon_hooks"] = mod
        antenv.axon_hooks = mod

        from trn_agent_boot.trn_boot import _ntff_profile_via_ctypes

        hook = _ntff_profile_via_ctypes("/opt/axon/libaxon_pjrt.so")
        mod.set_axon_ntff_profile_hook(hook)
    except Exception:
        pass


def kernel(**inputs):
    global LAST_RESULTS
    G = np.asarray(inputs["geneset_features"], dtype=np.float32)
    logits = np.asarray(inputs["attn_logits"], dtype=np.float32)
    flat_idx = np.asarray(inputs["flat_idx"]).astype(np.int64)
    seg = np.asarray(inputs["segment_ids"]).astype(np.int64)
    T = logits.shape[0]

    # Host-side layout prep: scatter exp(logits) into the sparse aggregation
    # matrix (member sets are sampled without replacement, so (idx, seg) pairs
    # are unique within a set and the fancy assignment is collision-free).
    e32 = np.exp(logits)
    W = np.zeros((NUM_GENESETS, NUM_SETS), dtype=ml_dtypes.bfloat16)
    W[flat_idx, seg] = e32.astype(ml_dtypes.bfloat16)

    # Padded per-set logit columns; device computes denominators from these.
    sizes = np.bincount(seg, minlength=NUM_SETS)
    starts = np.concatenate([[0], np.cumsum(sizes)[:-1]])
    pos = np.arange(T) - starts[seg]
    plogT = np.full((PAD_SLOTS, NUM_SETS), NEG_FILL, dtype=np.float32)
    plogT[pos, seg] = logits

    Gb = G.astype(ml_dtypes.bfloat16)

    GbT = np.ascontiguousarray(Gb.T)  # (8192, 1024)
    in_maps = []
    for c in range(N_CORES):
        bg, sg = divmod(c, SG)
        gt = GbT[:, bg * B_C : (bg + 1) * B_C].reshape(K_TILES, P, B_C)
        w = W[:, sg * S_C : (sg + 1) * S_C].reshape(K_TILES, P, S_C)
        # flat per-partition-contiguous layout: gw[p, k*FD + c] so one DMA
        # can move multiple K-tiles as large contiguous descriptor rows
        gw = (
            np.concatenate([gt, w], axis=2)  # (K_TILES, P, FD)
            .transpose(1, 0, 2)
            .reshape(P, K_TILES * (B_C + S_C))
        )
        # sets-on-partitions layout: plog[s_local, j*128+t] = logit slot t
        # of set (sg*S_C + j*128 + s_local)
        chunk = plogT[:, sg * S_C : (sg + 1) * S_C]  # (slots, S_C)
        plog = np.ascontiguousarray(
            chunk.reshape(PAD_SLOTS, S_C // P, P).transpose(2, 1, 0)
        )  # (P, SUBT, PAD_SLOTS)
        in_maps.append({"gw": np.ascontiguousarray(gw), "plog": plog})

    from concourse.bass_utils import run_bass_kernel_spmd

    _ensure_ntff_hook()
    nc = _get_program()
    res = run_bass_kernel_spmd(nc, in_maps, core_ids=list(range(N_CORES)))
    LAST_RESULTS = res

    out = np.empty((BATCH, NUM_SETS), dtype=np.float32)
    for c in range(N_CORES):
        bg, sg = divmod(c, SG)
        # device out is [P, SUBT, B_C] bf16 with set s = j*128 + p
        ot = np.asarray(res.results[c]["out"]).astype(np.float32)
        block = ot.transpose(1, 0, 2).reshape(S_C, B_C)
        out[bg * B_C : (bg + 1) * B_C, sg * S_C : (sg + 1) * S_C] = block.T
    return out

